# revision 26
# baseline (speedup 1.0000x reference)
"""Barlow Twins loss on 8 trn2 NeuronCores — hand-scheduled Bass kernel.

Math: with A = normalize(z_a), B = normalize(z_b) (per-column, ddof=1) and
c = A.T @ B / N:

    loss = lam * sum(c**2) + sum_d [ (c_dd - 1)**2 - lam * c_dd**2 ]
    sum(c**2) = tr((A A.T)(B B.T)) / N^2      (Gram matrices are [N, N])

Ga = A A.T is separable over column shards (Ga = sum_cores A_i A_i.T), so each
core computes partial [256, 256] Grams over its 1024-column slice via PE
matmuls on bf16-normalized tiles, plus raw per-column dots r_d = sum_n a*b
(host corrects: c_dd = (r_d - N mu_a mu_b) * istd_a * istd_b / N) and
per-column mean/var.  The host reduces the 8 partials in float64.

The device program is raw per-engine code (no Tile): inputs arrive as bf16
[1024, 256] transposed slices (d = 8p + i), two half-DMAs per tensor on the
two HWDGE rings (sync ring = z_a, scalar ring = z_b); per-half stats chains
(vector reduces, scalar-engine squares/sqrt, small [128,4] vector ops);
normalized bf16 tiles feed 32 PE matmuls accumulated in 4 PSUM banks; diag
products run on gpsimd.  PE is pre-warmed with dummy matmuls so the real ones
run at high p-state.
"""

import numpy as np

N = 256
D = 8192
NCORES = 8
D_LOCAL = D // NCORES  # 1024
P = 128
NT = D_LOCAL // P  # 8 tiles per tensor per core
NH = NT // 2  # tiles per half
LAMBDA = 0.005

_CACHE: dict = {}

# norm engine assignment per (tensor, tile): scalar engine does half 0
# (it is idle while the vector engine runs the stats chains), vector does
# half 1 after its chains finish (no gpsimd nb bias needed for those).
DVE_NORMS = {(t, i) for t in "ab" for i in (4, 5, 6, 7)}
N_DUMMY_MM = 10


def _build_program(ev_in=None):
    ev_in = ev_in or {}
    import concourse.bacc as bacc
    from concourse import mybir

    f32 = mybir.dt.float32
    bf16 = mybir.dt.bfloat16
    Alu = mybir.AluOpType
    Act = mybir.ActivationFunctionType
    X = mybir.AxisListType.X

    nc = bacc.Bacc("TRN2", target_bir_lowering=False, debug=False)

    za_t = nc.dram_tensor("za_t", [D_LOCAL, N], bf16, kind="ExternalInput").ap()
    zb_t = nc.dram_tensor("zb_t", [D_LOCAL, N], bf16, kind="ExternalInput").ap()
    ga = nc.dram_tensor("ga", [2, P, N], f32, kind="ExternalOutput").ap()
    gb = nc.dram_tensor("gb", [2, P, N], f32, kind="ExternalOutput").ap()
    qd = nc.dram_tensor("qd", [P, NT], f32, kind="ExternalOutput").ap()
    # per-tensor stats: [..., 0] = mean, [..., 1] = biased var
    st_a = nc.dram_tensor("st_a", [P, NT, 2], f32, kind="ExternalOutput").ap()
    st_b = nc.dram_tensor("st_b", [P, NT, 2], f32, kind="ExternalOutput").ap()

    src = {
        "a": za_t.rearrange("(p i) n -> p (i n)", i=NT),
        "b": zb_t.rearrange("(p i) n -> p (i n)", i=NT),
    }

    # ---- SBUF / PSUM ----
    raw = {t: nc.alloc_sbuf_tensor(f"raw_{t}", [P, NT, N], bf16).ap() for t in "ab"}
    zn = {t: nc.alloc_sbuf_tensor(f"zn_{t}", [P, NT, N], bf16).ap() for t in "ab"}
    prod = nc.alloc_sbuf_tensor("prod", [P, NT, N], bf16).ap()
    bnst = {t: nc.alloc_sbuf_tensor(f"bnst_{t}", [P, NT, 6], f32).ap() for t in "ab"}
    smv = {t: nc.alloc_sbuf_tensor(f"smv_{t}", [P, NT, 2], f32).ap() for t in "ab"}
    iv = {t: nc.alloc_sbuf_tensor(f"iv_{t}", [P, NT], f32).ap() for t in "ab"}
    sd = {t: nc.alloc_sbuf_tensor(f"sd_{t}", [P, NT], f32).ap() for t in "ab"}
    nbm = {t: nc.alloc_sbuf_tensor(f"nbm_{t}", [P, NT], f32).ap() for t in "ab"}
    nb = {t: nc.alloc_sbuf_tensor(f"nb_{t}", [P, NT], f32).ap() for t in "ab"}
    q_sb = nc.alloc_sbuf_tensor("q_sb", [P, NT], f32).ap()
    g_sb = {t: nc.alloc_sbuf_tensor(f"g_sb_{t}", [P, 2, N], f32).ap() for t in "ab"}
    scr1 = nc.alloc_sbuf_tensor("scr1", [P, 1], f32).ap()
    gps = {
        t: [nc.alloc_psum_tensor(f"g_ps_{t}{m}", [P, N], f32).ap() for m in range(2)]
        for t in "ab"
    }
    dummy_ps = nc.alloc_psum_tensor("dummy_ps", [P, N], f32).ap()
    dummy_sb = nc.alloc_sbuf_tensor("dummy_sb", [P, N], bf16).ap()

    def mn(t, i):  # [P, 1] mean column for tile i
        return smv[t][:, i, 0:1]

    # ---- semaphores ----
    # One rolling "chain" semaphore per compute engine; cross-engine deps
    # wait on the producer engine's chain value at the producer's index.
    sem = {
        name: nc.alloc_semaphore(name)
        for name in (
            "da0", "da1", "db0", "db1",
            "vch", "ach", "pch",
            "mma", "mmb", "dout_s", "dout_a",
        )
    }
    dmas = {("a", 0): sem["da0"], ("a", 1): sem["da1"],
            ("b", 0): sem["db0"], ("b", 1): sem["db1"]}
    mms = {"a": sem["mma"], "b": sem["mmb"]}

    cnt = {"v": 0, "a": 0, "p": 0}
    chain = {"v": sem["vch"], "a": sem["ach"], "p": sem["pch"]}
    ev = {}

    def em(ek, ins, event=None):
        ins._wait_ge(chain[ek], cnt[ek])
        ins.then_inc(chain[ek], 1)
        cnt[ek] += 1
        if event:
            ev[event] = (ek, cnt[ek])
        return ins

    def wait_ev(eng, ek, event):
        val = ev_in.get(event, (ek, 0))[1]
        eng.wait_ge(chain[ek], val)

    def tsl(h):  # tile slice of half h
        return slice(h * NH, (h + 1) * NH)

    # PE consumption order (tile ready-time order)
    PE_SCHED = [("a", 0), ("a", 1), ("a", 2), ("a", 3), ("b", 0), ("b", 1),
                ("b", 2), ("b", 3), ("a", 4), ("a", 5), ("a", 6), ("a", 7),
                ("b", 4), ("b", 5), ("b", 6), ("b", 7)]
    first_tile = {"a": ("a", 0), "b": ("b", 0)}
    last_tile = {"a": ("a", 7), "b": ("b", 7)}

    with nc.Block() as block:

        @block.vector
        def _(vector):
            def chain_th(t, h):
                nc.vector.wait_ge(dmas[(t, 0)], 16)
                for i in range(h * NH, (h + 1) * NH):
                    em("v", nc.vector.bn_stats(
                        bnst[t][:, i, :], raw[t][:, i, :]))
                for i in range(h * NH, (h + 1) * NH):
                    em("v", nc.vector.bn_aggr(
                        smv[t][:, i, :], bnst[t][:, i, :]),
                        event=f"bn_{t}{h}" if i == (h + 1) * NH - 1 else None)
                em("v", nc.vector.reciprocal(
                    iv[t][:, tsl(h)], smv[t][:, tsl(h), 1]),
                    event=f"iv_{t}{h}")

            def vnorm(t, i):
                wait_ev(nc.vector, "a", f"istd_{t}{i // NH}")
                em("v", nc.vector.tensor_scalar(
                    out=zn[t][:, i, :], in0=raw[t][:, i, :],
                    scalar1=mn(t, i), scalar2=sd[t][:, i : i + 1],
                    op0=Alu.subtract, op1=Alu.mult,
                ), event=f"norm_{t}{i}")

            chain_th("a", 0)
            chain_th("b", 0)
            chain_th("a", 1)
            chain_th("b", 1)
            for t, i in sorted(DVE_NORMS):
                vnorm(t, i)
            # diag reduces (prods from gpsimd)
            for h in range(2):
                wait_ev(nc.vector, "p", f"prod{h}")
                em("v", nc.vector.reduce_sum(
                    q_sb[:, tsl(h)], prod[:, tsl(h), :], axis=X),
                    event=f"qred{h}" if h == 1 else None)

        @block.scalar
        def _(scalar):
            nc.scalar.dma_start(
                raw["b"].rearrange("p i n -> p (i n)"), src["b"][:]
            ).then_inc(sem["db0"], 16)
            # preload ACT tables (Sqrt + Identity) while DMAs fly
            em("a", nc.scalar.sqrt(scr1[:], nc.const_aps.scalar_like(1.0, scr1)))
            em("a", nc.scalar.activation(scr1[:], scr1[:], Act.Identity))
            kB = (N - 1.0) / N
            for t in "ab":
                # half-0 sqrt + norms (tiles 0..3)
                wait_ev(nc.scalar, "v", f"iv_{t}0")
                em("a", nc.scalar.activation(
                    sd[t][:, tsl(0)], iv[t][:, tsl(0)], Act.Sqrt, scale=kB),
                    event=f"istd_{t}0")
                wait_ev(nc.scalar, "p", f"nb_{t}0")
                for i in range(0, NH):
                    em("a", nc.scalar.activation(
                        zn[t][:, i, :], raw[t][:, i, :], Act.Identity,
                        bias=nb[t][:, i : i + 1], scale=sd[t][:, i : i + 1],
                    ), event=f"norm_{t}{i}")
            # half-1 sqrts (vector engine does those norms)
            for t in "ab":
                wait_ev(nc.scalar, "v", f"iv_{t}1")
                em("a", nc.scalar.activation(
                    sd[t][:, tsl(1)], iv[t][:, tsl(1)], Act.Sqrt, scale=kB),
                    event=f"istd_{t}1")
            # psum copies + gb out on this ring
            for t in "ab":
                nc.scalar.wait_ge(mms[t], 2)
                em("a", nc.scalar.copy(
                    g_sb[t][:, 0, :], gps[t][0][:]), event=f"cp0_{t}")
                em("a", nc.scalar.copy(
                    g_sb[t][:, 1, :], gps[t][1][:]), event=f"cp1_{t}")
            wait_ev(nc.scalar, "a", "cp1_b")
            nc.scalar.dma_start(
                gb.rearrange("m p n -> p m n"), g_sb["b"][:]
            ).then_inc(sem["dout_a"], 16)
            nc.scalar.wait_ge(sem["dout_a"], 16)

        @block.gpsimd
        def _(gpsimd):
            def nbchain(t):
                c = slice(0, NH)
                wait_ev(nc.gpsimd, "a", f"istd_{t}0")
                em("p", nc.gpsimd.tensor_tensor(
                    nbm[t][:, c], smv[t][:, c, 0], sd[t][:, c], op=Alu.mult))
                em("p", nc.gpsimd.tensor_scalar_mul(
                    nb[t][:, c], nbm[t][:, c], -1.0), event=f"nb_{t}0")

            em("p", nc.gpsimd.memset(dummy_sb[:], 0.0), event="dumz")
            nc.gpsimd.wait_ge(sem["da0"], 16)
            nc.gpsimd.wait_ge(sem["db0"], 16)
            em("p", nc.gpsimd.tensor_tensor(
                prod[:, tsl(0), :], raw["a"][:, tsl(0), :],
                raw["b"][:, tsl(0), :], op=Alu.mult), event="prod0")
            nbchain("a")
            nbchain("b")
            em("p", nc.gpsimd.tensor_tensor(
                prod[:, tsl(1), :], raw["a"][:, tsl(1), :],
                raw["b"][:, tsl(1), :], op=Alu.mult), event="prod1")
        @block.tensor
        def _(tensor):
            # p-state warmup: dummy matmuls on zeroed scratch, gated on input
            # DMA arrival so the PE is still hot when the real matmuls start
            wait_ev(nc.tensor, "p", "dumz")
            nc.tensor.wait_ge(sem["da0"], 16)
            for _i in range(N_DUMMY_MM):
                nc.tensor.matmul(
                    dummy_ps[:], lhsT=dummy_sb[:, 0:P], rhs=dummy_sb[:],
                    start=True, stop=True, skip_group_check=True,
                )
            for t, i in PE_SCHED:
                wait_ev(nc.tensor, "v" if (t, i) in DVE_NORMS else "a",
                        f"norm_{t}{i}")
                first = (t, i) == first_tile[t]
                last = (t, i) == last_tile[t]
                for m in range(2):
                    ins = nc.tensor.matmul(
                        gps[t][m][:], lhsT=zn[t][:, i, m * P : (m + 1) * P],
                        rhs=zn[t][:, i, :], start=first, stop=last,
                    )
                    if last:
                        ins.then_inc(mms[t], 1)

        @block.sync
        def _(sync):
            nc.sync.dma_start(raw["a"].rearrange("p i n -> p (i n)"), src["a"][:]).then_inc(sem["da0"], 16)
            # outputs: qd last (qred1 is the latest producer) to avoid
            # head-of-line blocking of the ga DMA on this ring
            wait_ev(nc.sync, "v", "bn_a1")
            nc.sync.dma_start(st_a[:], smv["a"][:]).then_inc(sem["dout_s"], 16)
            wait_ev(nc.sync, "v", "bn_b1")
            nc.sync.dma_start(st_b[:], smv["b"][:]).then_inc(sem["dout_s"], 16)
            wait_ev(nc.sync, "a", "cp0_a")
            wait_ev(nc.sync, "a", "cp1_a")
            nc.sync.dma_start(
                ga.rearrange("m p n -> p m n"), g_sb["a"][:]
            ).then_inc(sem["dout_s"], 16)
            wait_ev(nc.sync, "v", "qred1")
            nc.sync.dma_start(qd[:], q_sb[:]).then_inc(sem["dout_s"], 16)
            nc.sync.wait_ge(sem["dout_s"], 64)

    nc.compile()
    return nc, ev


def _get_program():
    if "nc" not in _CACHE:
        _, ev = _build_program()       # pass 1: record event chain indices
        _CACHE["nc"], _ = _build_program(ev)  # pass 2: real wait values
    return _CACHE["nc"]


LAST_RESULT = None


def kernel(z_a: np.ndarray, z_b: np.ndarray) -> np.ndarray:
    global LAST_RESULT
    import ml_dtypes

    from concourse.bass_utils import run_bass_kernel_spmd

    z_a = np.asarray(z_a, dtype=np.float32)
    z_b = np.asarray(z_b, dtype=np.float32)
    assert z_a.shape == (N, D) and z_b.shape == (N, D)

    nc = _get_program()

    bf = ml_dtypes.bfloat16
    in_maps = []
    for c in range(NCORES):
        sl = slice(c * D_LOCAL, (c + 1) * D_LOCAL)
        in_maps.append(
            {
                "za_t": np.ascontiguousarray(z_a[:, sl].T.astype(bf)),
                "zb_t": np.ascontiguousarray(z_b[:, sl].T.astype(bf)),
            }
        )

    res = run_bass_kernel_spmd(nc, in_maps, core_ids=list(range(NCORES)))
    LAST_RESULT = res

    Ga = np.zeros((2 * P, N), dtype=np.float64)
    Gb = np.zeros((2 * P, N), dtype=np.float64)
    q = np.empty(D, dtype=np.float64)  # c_dd * N
    for c in range(NCORES):
        out = res.results[c]
        Ga += out["ga"].reshape(2 * P, N).astype(np.float64)
        Gb += out["gb"].reshape(2 * P, N).astype(np.float64)
        sta = out["st_a"].astype(np.float64)
        stb = out["st_b"].astype(np.float64)
        mean_a, var_a = sta[:, :, 0], sta[:, :, 1] * (N / (N - 1.0))
        mean_b, var_b = stb[:, :, 0], stb[:, :, 1] * (N / (N - 1.0))
        r = out["qd"].astype(np.float64)  # [P, NT] raw sum_n a*b at (p, i)
        qc = (r - N * mean_a * mean_b) / np.sqrt(var_a * var_b)
        q[c * D_LOCAL : (c + 1) * D_LOCAL] = qc.reshape(D_LOCAL)

    sum_c2 = float((Ga * Gb).sum()) / (N * N)  # sum over ALL (d, e) of c^2
    cdd = q / N
    loss = (
        LAMBDA * (sum_c2 - float((cdd * cdd).sum()))
        + float(((cdd - 1.0) ** 2).sum())
    )
    return np.float32(loss)


if __name__ == "__main__":
    rng = np.random.default_rng(0)
    za = rng.standard_normal((N, D), dtype=np.float32)
    zb = rng.standard_normal((N, D), dtype=np.float32)
    out = kernel(z_a=za, z_b=zb)
    print("kernel output:", out)


# revision 27
# speedup vs baseline: 1.0044x; 1.0044x over previous
"""Barlow Twins loss on 8 trn2 NeuronCores — hand-scheduled Bass kernel.

Math: with A = normalize(z_a), B = normalize(z_b) (per-column, ddof=1) and
c = A.T @ B / N:

    loss = lam * sum(c**2) + sum_d [ (c_dd - 1)**2 - lam * c_dd**2 ]
    sum(c**2) = tr((A A.T)(B B.T)) / N^2      (Gram matrices are [N, N])

Ga = A A.T is separable over column shards (Ga = sum_cores A_i A_i.T), so each
core computes partial [256, 256] Grams over its 1024-column slice via PE
matmuls on bf16-normalized tiles, plus raw per-column dots r_d = sum_n a*b
(host corrects: c_dd = (r_d - N mu_a mu_b) * istd_a * istd_b / N) and
per-column mean/var.  The host reduces the 8 partials in float64.

The device program is raw per-engine code (no Tile): inputs arrive as bf16
[1024, 256] transposed slices (d = 8p + i), two half-DMAs per tensor on the
two HWDGE rings (sync ring = z_a, scalar ring = z_b); per-half stats chains
(vector reduces, scalar-engine squares/sqrt, small [128,4] vector ops);
normalized bf16 tiles feed 32 PE matmuls accumulated in 4 PSUM banks; diag
products run on gpsimd.  PE is pre-warmed with dummy matmuls so the real ones
run at high p-state.
"""

import numpy as np

N = 256
D = 8192
NCORES = 8
D_LOCAL = D // NCORES  # 1024
P = 128
NT = D_LOCAL // P  # 8 tiles per tensor per core
NH = NT // 2  # tiles per half
LAMBDA = 0.005

_CACHE: dict = {}

# norm engine assignment per (tensor, tile): scalar engine does half 0
# (it is idle while the vector engine runs the stats chains), vector does
# half 1 after its chains finish (no gpsimd nb bias needed for those).
DVE_NORMS = {("b", 4), ("b", 5), ("b", 6), ("b", 7)}
N_DUMMY_MM = 10


def _build_program(ev_in=None):
    ev_in = ev_in or {}
    import concourse.bacc as bacc
    from concourse import mybir

    f32 = mybir.dt.float32
    bf16 = mybir.dt.bfloat16
    Alu = mybir.AluOpType
    Act = mybir.ActivationFunctionType
    X = mybir.AxisListType.X

    nc = bacc.Bacc("TRN2", target_bir_lowering=False, debug=False)

    za_t = nc.dram_tensor("za_t", [D_LOCAL, N], bf16, kind="ExternalInput").ap()
    zb_t = nc.dram_tensor("zb_t", [D_LOCAL, N], bf16, kind="ExternalInput").ap()
    ga = nc.dram_tensor("ga", [2, P, N], f32, kind="ExternalOutput").ap()
    gb = nc.dram_tensor("gb", [2, P, N], f32, kind="ExternalOutput").ap()
    qd = nc.dram_tensor("qd", [P, NT], f32, kind="ExternalOutput").ap()
    # per-tensor stats: [..., 0] = mean, [..., 1] = biased var
    st_a = nc.dram_tensor("st_a", [P, NT, 2], f32, kind="ExternalOutput").ap()
    st_b = nc.dram_tensor("st_b", [P, NT, 2], f32, kind="ExternalOutput").ap()

    src = {
        "a": za_t.rearrange("(p i) n -> p (i n)", i=NT),
        "b": zb_t.rearrange("(p i) n -> p (i n)", i=NT),
    }

    # ---- SBUF / PSUM ----
    raw = {t: nc.alloc_sbuf_tensor(f"raw_{t}", [P, NT, N], bf16).ap() for t in "ab"}
    zn = {t: nc.alloc_sbuf_tensor(f"zn_{t}", [P, NT, N], bf16).ap() for t in "ab"}
    prod = nc.alloc_sbuf_tensor("prod", [P, NT, N], bf16).ap()
    bnst = {t: nc.alloc_sbuf_tensor(f"bnst_{t}", [P, NT, 6], f32).ap() for t in "ab"}
    smv = {t: nc.alloc_sbuf_tensor(f"smv_{t}", [P, NT, 2], f32).ap() for t in "ab"}
    iv = {t: nc.alloc_sbuf_tensor(f"iv_{t}", [P, NT], f32).ap() for t in "ab"}
    sd = {t: nc.alloc_sbuf_tensor(f"sd_{t}", [P, NT], f32).ap() for t in "ab"}
    nbm = {t: nc.alloc_sbuf_tensor(f"nbm_{t}", [P, NT], f32).ap() for t in "ab"}
    nb = {t: nc.alloc_sbuf_tensor(f"nb_{t}", [P, NT], f32).ap() for t in "ab"}
    q_sb = nc.alloc_sbuf_tensor("q_sb", [P, NT], f32).ap()
    g_sb = {t: nc.alloc_sbuf_tensor(f"g_sb_{t}", [P, 2, N], f32).ap() for t in "ab"}
    scr1 = nc.alloc_sbuf_tensor("scr1", [P, 1], f32).ap()
    gps = {
        t: [nc.alloc_psum_tensor(f"g_ps_{t}{m}", [P, N], f32).ap() for m in range(2)]
        for t in "ab"
    }
    dummy_ps = nc.alloc_psum_tensor("dummy_ps", [P, N], f32).ap()
    dummy_sb = nc.alloc_sbuf_tensor("dummy_sb", [P, N], bf16).ap()

    def mn(t, i):  # [P, 1] mean column for tile i
        return smv[t][:, i, 0:1]

    # ---- semaphores ----
    # One rolling "chain" semaphore per compute engine; cross-engine deps
    # wait on the producer engine's chain value at the producer's index.
    sem = {
        name: nc.alloc_semaphore(name)
        for name in (
            "da0", "da1", "db0", "db1",
            "vch", "ach", "pch",
            "mma", "mmb", "dout_s", "dout_a",
        )
    }
    dmas = {("a", 0): sem["da0"], ("a", 1): sem["da1"],
            ("b", 0): sem["db0"], ("b", 1): sem["db1"]}
    mms = {"a": sem["mma"], "b": sem["mmb"]}

    cnt = {"v": 0, "a": 0, "p": 0}
    chain = {"v": sem["vch"], "a": sem["ach"], "p": sem["pch"]}
    ev = {}

    def em(ek, ins, event=None):
        ins._wait_ge(chain[ek], cnt[ek])
        ins.then_inc(chain[ek], 1)
        cnt[ek] += 1
        if event:
            ev[event] = (ek, cnt[ek])
        return ins

    def wait_ev(eng, ek, event):
        val = ev_in.get(event, (ek, 0))[1]
        eng.wait_ge(chain[ek], val)

    def tsl(h):  # tile slice of half h
        return slice(h * NH, (h + 1) * NH)

    # PE consumption order (tile ready-time order)
    PE_SCHED = [("a", 0), ("a", 1), ("a", 2), ("a", 3), ("b", 0), ("b", 1),
                ("b", 2), ("b", 3), ("a", 4), ("a", 5), ("a", 6), ("a", 7),
                ("b", 4), ("b", 5), ("b", 6), ("b", 7)]
    first_tile = {"a": ("a", 0), "b": ("b", 0)}
    last_tile = {"a": ("a", 7), "b": ("b", 7)}

    with nc.Block() as block:

        @block.vector
        def _(vector):
            def chain_th(t, h):
                nc.vector.wait_ge(dmas[(t, h)], 16)
                for i in range(h * NH, (h + 1) * NH):
                    em("v", nc.vector.bn_stats(
                        bnst[t][:, i, :], raw[t][:, i, :]))
                for i in range(h * NH, (h + 1) * NH):
                    em("v", nc.vector.bn_aggr(
                        smv[t][:, i, :], bnst[t][:, i, :]),
                        event=f"bn_{t}{h}" if i == (h + 1) * NH - 1 else None)
                em("v", nc.vector.reciprocal(
                    iv[t][:, tsl(h)], smv[t][:, tsl(h), 1]),
                    event=f"iv_{t}{h}")

            def vnorm(t, i):
                wait_ev(nc.vector, "a", f"istd_{t}{i // NH}")
                em("v", nc.vector.tensor_scalar(
                    out=zn[t][:, i, :], in0=raw[t][:, i, :],
                    scalar1=mn(t, i), scalar2=sd[t][:, i : i + 1],
                    op0=Alu.subtract, op1=Alu.mult,
                ), event=f"norm_{t}{i}")

            chain_th("a", 0)
            chain_th("b", 0)
            chain_th("a", 1)
            chain_th("b", 1)
            for t, i in sorted(DVE_NORMS):
                vnorm(t, i)
            # diag reduces (prods from gpsimd)
            for h in range(2):
                wait_ev(nc.vector, "p", f"prod{h}")
                em("v", nc.vector.reduce_sum(
                    q_sb[:, tsl(h)], prod[:, tsl(h), :], axis=X),
                    event=f"qred{h}" if h == 1 else None)

        @block.scalar
        def _(scalar):
            fb = raw["b"].rearrange("p i n -> p (i n)")
            nc.scalar.dma_start(
                fb[:, 0 : NH * N], src["b"][:, 0 : NH * N]
            ).then_inc(sem["db0"], 16)
            nc.scalar.dma_start(
                fb[:, NH * N : NT * N], src["b"][:, NH * N : NT * N]
            ).then_inc(sem["db1"], 16)
            # preload ACT tables (Sqrt + Identity) while DMAs fly
            em("a", nc.scalar.sqrt(scr1[:], nc.const_aps.scalar_like(1.0, scr1)))
            em("a", nc.scalar.activation(scr1[:], scr1[:], Act.Identity))
            kB = (N - 1.0) / N
            for t in "ab":
                # half-0 sqrt + norms (tiles 0..3)
                wait_ev(nc.scalar, "v", f"iv_{t}0")
                em("a", nc.scalar.activation(
                    sd[t][:, tsl(0)], iv[t][:, tsl(0)], Act.Sqrt, scale=kB),
                    event=f"istd_{t}0")
                wait_ev(nc.scalar, "p", f"nb_{t}0")
                for i in range(0, NH):
                    em("a", nc.scalar.activation(
                        zn[t][:, i, :], raw[t][:, i, :], Act.Identity,
                        bias=nb[t][:, i : i + 1], scale=sd[t][:, i : i + 1],
                    ), event=f"norm_{t}{i}")
            # a half-1: sqrt + norms here (vector engine is still in chains)
            wait_ev(nc.scalar, "v", "iv_a1")
            em("a", nc.scalar.activation(
                sd["a"][:, tsl(1)], iv["a"][:, tsl(1)], Act.Sqrt, scale=kB),
                event="istd_a1")
            wait_ev(nc.scalar, "p", "nb_a1")
            for i in range(NH, NT):
                em("a", nc.scalar.activation(
                    zn["a"][:, i, :], raw["a"][:, i, :], Act.Identity,
                    bias=nb["a"][:, i : i + 1], scale=sd["a"][:, i : i + 1],
                ), event=f"norm_a{i}")
            # b half-1 sqrt (vector engine does those norms)
            wait_ev(nc.scalar, "v", "iv_b1")
            em("a", nc.scalar.activation(
                sd["b"][:, tsl(1)], iv["b"][:, tsl(1)], Act.Sqrt, scale=kB),
                event="istd_b1")
            # psum copies + gb out on this ring
            for t in "ab":
                nc.scalar.wait_ge(mms[t], 2)
                em("a", nc.scalar.copy(
                    g_sb[t][:, 0, :], gps[t][0][:]), event=f"cp0_{t}")
                em("a", nc.scalar.copy(
                    g_sb[t][:, 1, :], gps[t][1][:]), event=f"cp1_{t}")
            wait_ev(nc.scalar, "a", "cp1_b")
            nc.scalar.dma_start(
                gb.rearrange("m p n -> p m n"), g_sb["b"][:]
            ).then_inc(sem["dout_a"], 16)
            nc.scalar.wait_ge(sem["dout_a"], 16)

        @block.gpsimd
        def _(gpsimd):
            def nbchain(t, h):
                c = tsl(h)
                wait_ev(nc.gpsimd, "a", f"istd_{t}{h}")
                em("p", nc.gpsimd.tensor_tensor(
                    nbm[t][:, c], smv[t][:, c, 0], sd[t][:, c], op=Alu.mult))
                em("p", nc.gpsimd.tensor_scalar_mul(
                    nb[t][:, c], nbm[t][:, c], -1.0), event=f"nb_{t}{h}")

            em("p", nc.gpsimd.memset(dummy_sb[:], 0.0), event="dumz")
            nc.gpsimd.wait_ge(sem["da0"], 16)
            nc.gpsimd.wait_ge(sem["db0"], 16)
            em("p", nc.gpsimd.tensor_tensor(
                prod[:, tsl(0), :], raw["a"][:, tsl(0), :],
                raw["b"][:, tsl(0), :], op=Alu.mult), event="prod0")
            nbchain("a", 0)
            nbchain("b", 0)
            nbchain("a", 1)
            nc.gpsimd.wait_ge(sem["da1"], 16)
            nc.gpsimd.wait_ge(sem["db1"], 16)
            em("p", nc.gpsimd.tensor_tensor(
                prod[:, tsl(1), :], raw["a"][:, tsl(1), :],
                raw["b"][:, tsl(1), :], op=Alu.mult), event="prod1")
        @block.tensor
        def _(tensor):
            # p-state warmup: dummy matmuls on zeroed scratch, gated on input
            # DMA arrival so the PE is still hot when the real matmuls start
            wait_ev(nc.tensor, "p", "dumz")
            nc.tensor.wait_ge(sem["da0"], 16)
            for _i in range(N_DUMMY_MM):
                nc.tensor.matmul(
                    dummy_ps[:], lhsT=dummy_sb[:, 0:P], rhs=dummy_sb[:],
                    start=True, stop=True, skip_group_check=True,
                )
            for t, i in PE_SCHED:
                wait_ev(nc.tensor, "v" if (t, i) in DVE_NORMS else "a",
                        f"norm_{t}{i}")
                first = (t, i) == first_tile[t]
                last = (t, i) == last_tile[t]
                for m in range(2):
                    ins = nc.tensor.matmul(
                        gps[t][m][:], lhsT=zn[t][:, i, m * P : (m + 1) * P],
                        rhs=zn[t][:, i, :], start=first, stop=last,
                    )
                    if last:
                        ins.then_inc(mms[t], 1)

        @block.sync
        def _(sync):
            fa = raw["a"].rearrange("p i n -> p (i n)")
            nc.sync.dma_start(
                fa[:, 0 : NH * N], src["a"][:, 0 : NH * N]
            ).then_inc(sem["da0"], 16)
            nc.sync.dma_start(
                fa[:, NH * N : NT * N], src["a"][:, NH * N : NT * N]
            ).then_inc(sem["da1"], 16)
            # outputs: qd last (qred1 is the latest producer) to avoid
            # head-of-line blocking of the ga DMA on this ring
            wait_ev(nc.sync, "v", "bn_a1")
            nc.sync.dma_start(st_a[:], smv["a"][:]).then_inc(sem["dout_s"], 16)
            wait_ev(nc.sync, "v", "bn_b1")
            nc.sync.dma_start(st_b[:], smv["b"][:]).then_inc(sem["dout_s"], 16)
            wait_ev(nc.sync, "a", "cp0_a")
            wait_ev(nc.sync, "a", "cp1_a")
            nc.sync.dma_start(
                ga.rearrange("m p n -> p m n"), g_sb["a"][:]
            ).then_inc(sem["dout_s"], 16)
            wait_ev(nc.sync, "v", "qred1")
            nc.sync.dma_start(qd[:], q_sb[:]).then_inc(sem["dout_s"], 16)
            nc.sync.wait_ge(sem["dout_s"], 64)

    nc.compile()
    return nc, ev


def _get_program():
    if "nc" not in _CACHE:
        _, ev = _build_program()       # pass 1: record event chain indices
        _CACHE["nc"], _ = _build_program(ev)  # pass 2: real wait values
    return _CACHE["nc"]


LAST_RESULT = None


def kernel(z_a: np.ndarray, z_b: np.ndarray) -> np.ndarray:
    global LAST_RESULT
    import ml_dtypes

    from concourse.bass_utils import run_bass_kernel_spmd

    z_a = np.asarray(z_a, dtype=np.float32)
    z_b = np.asarray(z_b, dtype=np.float32)
    assert z_a.shape == (N, D) and z_b.shape == (N, D)

    nc = _get_program()

    bf = ml_dtypes.bfloat16
    in_maps = []
    for c in range(NCORES):
        sl = slice(c * D_LOCAL, (c + 1) * D_LOCAL)
        in_maps.append(
            {
                "za_t": np.ascontiguousarray(z_a[:, sl].T.astype(bf)),
                "zb_t": np.ascontiguousarray(z_b[:, sl].T.astype(bf)),
            }
        )

    res = run_bass_kernel_spmd(nc, in_maps, core_ids=list(range(NCORES)))
    LAST_RESULT = res

    Ga = np.zeros((2 * P, N), dtype=np.float64)
    Gb = np.zeros((2 * P, N), dtype=np.float64)
    q = np.empty(D, dtype=np.float64)  # c_dd * N
    for c in range(NCORES):
        out = res.results[c]
        Ga += out["ga"].reshape(2 * P, N).astype(np.float64)
        Gb += out["gb"].reshape(2 * P, N).astype(np.float64)
        sta = out["st_a"].astype(np.float64)
        stb = out["st_b"].astype(np.float64)
        mean_a, var_a = sta[:, :, 0], sta[:, :, 1] * (N / (N - 1.0))
        mean_b, var_b = stb[:, :, 0], stb[:, :, 1] * (N / (N - 1.0))
        r = out["qd"].astype(np.float64)  # [P, NT] raw sum_n a*b at (p, i)
        qc = (r - N * mean_a * mean_b) / np.sqrt(var_a * var_b)
        q[c * D_LOCAL : (c + 1) * D_LOCAL] = qc.reshape(D_LOCAL)

    sum_c2 = float((Ga * Gb).sum()) / (N * N)  # sum over ALL (d, e) of c^2
    cdd = q / N
    loss = (
        LAMBDA * (sum_c2 - float((cdd * cdd).sum()))
        + float(((cdd - 1.0) ** 2).sum())
    )
    return np.float32(loss)


if __name__ == "__main__":
    rng = np.random.default_rng(0)
    za = rng.standard_normal((N, D), dtype=np.float32)
    zb = rng.standard_normal((N, D), dtype=np.float32)
    out = kernel(z_a=za, z_b=zb)
    print("kernel output:", out)
